# revision 16
# baseline (speedup 1.0000x reference)
"""CosyVoice2 attention (B=8, S=2048, H=896, 14Q/2KV GQA, RoPE, causal) as a
Trainium2 Bass/Tile kernel, data-parallel over batch across 8 NeuronCores.

Per-core program (one batch element per core, no collectives):
  - host supplies X^T [896, 2048] (hidden on partitions, 7 chunks of 128),
    weights in matmul-ready layouts, and RoPE cos/sin tables with the
    rotate-half sign folded in.
  - QKV projections run as fp32r matmuls (full PE rate at N>=256) with
    biases added by K=1 ones-row matmuls into the same PSUM group.
  - RoPE in [d, s] layout:  out = x.*cos4 + Pneg @ (x.*sinm4), where
    Pneg[i, i^32] = -1.
  - scores S^T[k, q] per (head-pair, k-chunk): two K=64 matmuls row-tiled to
    opposite PE array halves run concurrently (heads h and h+7).
  - softmax: one ACT exp per k-chunk ([128, 2, 512] PSUM pair -> bf16
    probs), scale=1/8 and a constant -4 bias folded in.  Diagonal chunks
    restrict the exp/scores/attnV access patterns to causally-reachable q
    columns; fully-masked columns are zeroed by a DVE memset and the
    triangle block gets a gpsimd affine_select.
  - attnV in bf16 with the denominator riding as a 65th ones-row.  At pair
    end the raw [65, 512] accumulators are evicted to bf16 SBUF slabs;
    normalization (denom broadcast matmul -> fast reciprocal -> DVE mul
    into the bf16 A^T slab) and the bf16 o_proj run as filler in the NEXT
    q-tile's attention span, so the pair boundary frees PSUM in two DVE
    copies instead of a long normalize chain.
  - the chunk loop is software-pipelined (scores/exp of chunk k+1 emitted
    before attnV of chunk k) with projection / normalize / o_proj work
    interleaved as filler so the PE stays dense and the HAM clock gate
    stays un-throttled.
"""

import os
import sys

for _p in ("/opt/trn_rl_repo", "/root/.axon_site/_ro/trn_rl_repo"):
    if _p not in sys.path and os.path.isdir(_p):
        sys.path.append(_p)

import contextlib

import numpy as np

import concourse.bacc as bacc
import concourse.mybir as mybir
import concourse.tile as tile
from concourse import bass_utils

B = 8
S = 2048
H = 896
NQ = 14
NKV = 2
D = 64
THETA = 1000000.0
P = 128
HC = H // P          # 7 hidden chunks
QT = 512             # q-tile width
NQT = S // QT        # 4 q-tiles
SC = S // P          # 16 seq chunks of 128
F32 = mybir.dt.float32
F32R = mybir.dt.float32r
BF16 = mybir.dt.bfloat16

# order in which Q head-dim chunks are projected; pair hp needs chunks
# {hp//2, (hp+7)//2}, so this order lets pair p start after the first
# REQ_CHUNKS[p] entries have been emitted.
Q_ORDER = [0, 3, 4, 1, 5, 2, 6]
REQ_CHUNKS = [2, 3, 4, 5, 6, 7, 7]

_CACHE = {}
LAST_RESULTS = None


def _build():
    nc = bacc.Bacc("TRN2", target_bir_lowering=False, debug=False, num_devices=8)

    xt_d = nc.dram_tensor("xt", [P, HC, S], F32R, kind="ExternalInput").ap()
    wq_d = nc.dram_tensor("wq", [P, HC, H], F32R, kind="ExternalInput").ap()
    wk_d = nc.dram_tensor("wk", [P, HC, P], F32R, kind="ExternalInput").ap()
    wk2_d = nc.dram_tensor("wk2", [P, HC, P], F32R, kind="ExternalInput").ap()
    wv_d = nc.dram_tensor("wv", [P, HC, P], F32R, kind="ExternalInput").ap()
    wo_d = nc.dram_tensor("wo", [P, HC, H], BF16, kind="ExternalInput").ap()
    bq_d = nc.dram_tensor("bq", [1, H], F32R, kind="ExternalInput").ap()
    bk_d = nc.dram_tensor("bk", [1, P], F32R, kind="ExternalInput").ap()
    bk2_d = nc.dram_tensor("bk2", [1, P], F32R, kind="ExternalInput").ap()
    bv_d = nc.dram_tensor("bv", [1, P], F32R, kind="ExternalInput").ap()
    cos_d = nc.dram_tensor("cos4", [P, S], F32R, kind="ExternalInput").ap()
    sin_d = nc.dram_tensor("sinm4", [P, S], F32R, kind="ExternalInput").ap()
    pneg_d = nc.dram_tensor("pneg", [P, P], F32R, kind="ExternalInput").ap()
    ones_d = nc.dram_tensor("onesr", [P, QT], F32R, kind="ExternalInput").ap()
    ident_d = nc.dram_tensor("ident", [P, P], F32R, kind="ExternalInput").ap()
    o_d = nc.dram_tensor("o", [P, SC, H], F32, kind="ExternalOutput").ap()

    with tile.TileContext(nc) as tc, contextlib.ExitStack() as ctx:
        const = ctx.enter_context(tc.tile_pool(name="const", bufs=1))
        work = ctx.enter_context(tc.tile_pool(name="work", bufs=2))
        ppool = ctx.enter_context(tc.tile_pool(name="ppool", bufs=3))
        rpool = ctx.enter_context(tc.tile_pool(name="rpool", bufs=2))
        npool = ctx.enter_context(tc.tile_pool(name="npool", bufs=3))
        psc = ctx.enter_context(tc.tile_pool(name="psc", bufs=2, space="PSUM"))
        pssc = ctx.enter_context(tc.tile_pool(name="pssc", bufs=2, space="PSUM"))
        psav = ctx.enter_context(tc.tile_pool(name="psav", bufs=1, space="PSUM"))

        # ---- tile-0 activations first on the DMA queue, weights in order
        # of first use, wo (the biggest, needed only in span 1) last ----
        xs0 = work.tile([P, HC, QT], F32R, tag="xs", name="xs0")
        cos0 = work.tile([P, QT], F32R, tag="cos_t", name="cos0")
        sin0 = work.tile([P, QT], F32R, tag="sin_t", name="sin0")

        wk_sb = const.tile([P, HC, P], F32R)
        wk2_sb = const.tile([P, HC, P], F32R)
        wv_sb = const.tile([P, HC, P], F32R)
        wo_sb = const.tile([P, HC, H], BF16)
        bq_sb = const.tile([1, H], F32R)
        bk_sb = const.tile([1, P], F32R)
        bk2_sb = const.tile([1, P], F32R)
        bv_sb = const.tile([1, P], F32R)
        pneg_sb = const.tile([P, P], F32R)
        ident_sb = const.tile([P, P], F32R)
        ones_sb = const.tile([P, QT], F32R)
        ones16 = const.tile([P, 64], BF16)
        bias_exp = const.tile([P, 1], F32)
        # first K-proj matmul needs only xs chunks 0-1 + wk; stream those
        # first, the rest in order of first use
        nc.sync.dma_start(out=xs0[:, 0:2, :], in_=xt_d[:, 0:2, 0:QT])
        nc.sync.dma_start(out=wk_sb, in_=wk_d)
        nc.sync.dma_start(out=bk_sb, in_=bk_d)
        nc.sync.dma_start(out=cos0, in_=cos_d[:, 0:QT])
        nc.sync.dma_start(out=sin0, in_=sin_d[:, 0:QT])
        nc.sync.dma_start(out=xs0[:, 2:HC, :], in_=xt_d[:, 2:HC, 0:QT])
        for dst, src in ((pneg_sb, pneg_d), (ones_sb, ones_d),
                         (bq_sb, bq_d), (wv_sb, wv_d), (bv_sb, bv_d),
                         (ident_sb, ident_d),
                         (wk2_sb, wk2_d), (bk2_sb, bk2_d), (wo_sb, wo_d)):
            nc.sync.dma_start(out=dst, in_=src)
        nc.vector.memset(bias_exp, -4.0)
        nc.vector.tensor_copy(ones16, ones_sb[:, 0:64])

        # K^T resident (two partition layouts) and V' resident
        kt = const.tile([P, S], F32R)    # parts 0-63 = kv0, 64-127 = kv1
        kt2 = const.tile([P, S], F32R)   # parts 0-63 = kv1, 64-127 = kv0
        vp = const.tile([P, SC, 130], BF16)  # [Vkv0 | ones | Vkv1 | ones]
        nc.vector.memset(vp[:, :, 64:65], 1.0)
        nc.vector.memset(vp[:, :, 129:130], 1.0)

        # tiny PE touches that pre-absorb weight-DMA waits; emitted right
        # before each weight's first real matmul use (fresh ring tile per
        # call so the proj slot is released promptly)
        _tn = [0]

        def touch(*tiles):
            _tn[0] += 1
            tch = psc.tile([1, 2], F32, tag="proj", name=f"tch{_tn[0]}")
            for t in tiles:
                ap = (t[0:1, 0, 0:1] if len(t.shape) == 3 else t[0:1, 0:1])
                if t.dtype != BF16:
                    ap = ap.bitcast(F32)
                nc.tensor.matmul(tch[:, 0:1], ap, ap, start=True, stop=True)

        def rope_into(dst_ap, src_psum, cos_t, sin_t, nm):
            """dst = src*cos4 + Pneg @ (src*sinm4); evicts psum with 1 read"""
            qe = rpool.tile([P, QT], F32R, tag="qe", name=f"qe_{nm}")
            nc.vector.tensor_copy(qe, src_psum)
            t1 = rpool.tile([P, QT], F32R, tag="t1", name=f"t1_{nm}")
            nc.vector.tensor_mul(t1, qe, cos_t)
            nc.vector.tensor_mul(qe, qe, sin_t)
            rp = psc.tile([P, QT], F32, tag="proj", name=f"rp_{nm}")
            nc.tensor.matmul(rp, pneg_sb, qe, start=True, stop=True)
            nc.vector.tensor_add(dst_ap, t1.bitcast(F32), rp)

        state = {}

        def gen_kv(t, preload=None):
            """DMAs + K (then deferred V, K2) projections for q-tile t."""
            tslice = slice(t * QT, (t + 1) * QT)
            if preload is not None:
                xs, cos_t, sin_t = preload
            else:
                xs = work.tile([P, HC, QT], F32R, tag="xs", name=f"xs{t}")
                nc.sync.dma_start(out=xs, in_=xt_d[:, :, tslice])
                cos_t = work.tile([P, QT], F32R, tag="cos_t", name=f"cos{t}")
                sin_t = work.tile([P, QT], F32R, tag="sin_t", name=f"sin{t}")
                nc.sync.dma_start(out=cos_t, in_=cos_d[:, tslice])
                nc.sync.dma_start(out=sin_t, in_=sin_d[:, tslice])
            qs = work.tile([P, HC, QT], F32R, tag="qs", name=f"qs{t}")
            state[t] = {"qs": qs, "xs": xs, "cos": cos_t, "sin": sin_t,
                        "qn": 0, "nn": 0, "k_done": False}

            def kproj(kdst, w_sb, b_sb, nm):
                kp = psc.tile([P, QT], F32, tag="proj", name=f"kp_{nm}")
                for c in range(HC):
                    nc.tensor.matmul(kp, w_sb[:, c, :], xs[:, c, :],
                                     start=(c == 0), stop=False)
                    if c == 3:
                        yield
                nc.tensor.matmul(kp, b_sb, ones_sb[0:1, :], start=False,
                                 stop=True)
                rope_into(kdst[:, tslice], kp, cos_t, sin_t, nm)
                yield

            if t == 0:
                touch(wk_sb, bk_sb, pneg_sb, ones_sb)
            yield from kproj(kt, wk_sb, bk_sb, f"k{t}")
            # V projection: V^T then PE-transpose per 128-chunk
            if t == 0:
                touch(wv_sb, bv_sb, ident_sb)
            vtp = psc.tile([P, QT], F32, tag="proj", name=f"vtp{t}")
            for c in range(HC):
                nc.tensor.matmul(vtp, wv_sb[:, c, :], xs[:, c, :],
                                 start=(c == 0), stop=False)
                if c == 3:
                    yield
            nc.tensor.matmul(vtp, bv_sb, ones_sb[0:1, :], start=False,
                             stop=True)
            vt_sb = rpool.tile([P, QT], F32R, tag="vt_sb", name=f"vt{t}")
            nc.vector.tensor_copy(vt_sb, vtp)
            for j in range(4):
                sc_i = t * 4 + j
                vtr = psc.tile([P, P], F32R, tag="proj", name=f"vtr{sc_i}")
                nc.tensor.transpose(vtr, vt_sb[:, j * P:(j + 1) * P], ident_sb)
                nc.vector.tensor_copy(vp[:, sc_i, 0:64], vtr[:, 0:64])
                nc.vector.tensor_copy(vp[:, sc_i, 65:129], vtr[:, 64:128])
                if j % 2 == 1:
                    yield
            state[t]["k_done"] = True
            if t == 0:
                touch(wk2_sb, bk2_sb)
            yield from kproj(kt2, wk2_sb, bk2_sb, f"k2{t}")

        def gen_q(t):
            """Q projection + rope for tile t, head-dim chunks in Q_ORDER."""
            st_ = state[t]
            xs, cos_t, sin_t, qs = st_["xs"], st_["cos"], st_["sin"], st_["qs"]
            if t == 0:
                touch(bq_sb)
            for c in Q_ORDER:
                wq_c = work.tile([P, HC, P], F32R, tag="wq_c", bufs=2,
                                 name=f"wq{t}_{c}")
                nc.sync.dma_start(out=wq_c, in_=wq_d[:, :, c * P:(c + 1) * P])
                qp = psc.tile([P, QT], F32, tag="proj", name=f"qp{t}_{c}")
                for hcc in range(HC):
                    nc.tensor.matmul(qp, wq_c[:, hcc, :], xs[:, hcc, :],
                                     start=(hcc == 0), stop=False)
                    if hcc == 3:
                        yield
                nc.tensor.matmul(qp, bq_sb[:, c * P:(c + 1) * P],
                                 ones_sb[0:1, :], start=False, stop=True)
                rope_into(qs[:, c, :], qp, cos_t, sin_t, f"q{t}_{c}")
                st_["qn"] += 1
                yield

        def norm_pair(t, hp):
            """Normalize pair hp of tile t from its raw bf16 slabs."""
            araw0, araw1 = state[t]["araw0"], state[t]["araw1"]
            aslab = state[t]["aslab"]
            for araw, rh in ((araw0, 0), (araw1, 64)):
                nm = f"n{t}_{hp}_{rh}"
                bc = psc.tile([64, QT], F32, tag="proj", name=f"bc{nm}")
                nc.tensor.matmul(bc, ones16[64:65, 0:64],
                                 araw[64:65, hp, :], start=True, stop=True)
                rc = npool.tile([64, QT], F32, tag="rc", bufs=2,
                                name=f"r{nm}")
                nc.vector.reciprocal_approx_fast(rc, bc)
                nc.vector.tensor_mul(aslab[rh:rh + 64, hp, :],
                                     araw[0:64, hp, :], rc)
                yield
            state[t]["nn"] += 1

        def gen_oproj(t):
            """bf16 o_proj of tile t (aslab must be normalized first)."""
            aslab = state[t]["aslab"]
            if t == 0:
                touch(wo_sb)
            for j in range(4):
                sc_i = t * 4 + j
                jsl = slice(j * P, (j + 1) * P)
                for n0, nw in ((0, 512), (512, 384)):
                    op = psc.tile([P, 512], F32, tag="proj",
                                  name=f"op{sc_i}_{n0}")
                    for c in range(HC):
                        # aslab chunk c must be normalized (emitted) first
                        while state[t]["nn"] <= c:
                            yield
                        nc.tensor.matmul(op[:, 0:nw], aslab[:, c, jsl],
                                         wo_sb[:, c, n0:n0 + nw],
                                         start=(c == 0), stop=(c == HC - 1))
                        if c == 3:
                            yield
                    osb = npool.tile([P, 512], F32, tag="osb", bufs=2,
                                     name=f"os{sc_i}_{n0}")
                    nc.vector.tensor_copy(osb[:, 0:nw], op[:, 0:nw])
                    nc.sync.dma_start(out=o_d[:, sc_i, n0:n0 + nw],
                                      in_=osb[:, 0:nw])
                    yield

        def attention_pair(t, hp):
            """Chunk-pipelined scores/softmax/attnV for head pair (hp, hp+7).

            Yields once per k-chunk; attnV matmuls for chunk kc are emitted
            after scores+exp of chunk kc+1 so the PE never waits on the ACT.
            Pair end evicts the raw accumulators (with denominator rows) to
            bf16 slabs; normalization happens next span.
            """
            qs = state[t]["qs"]
            araw0, araw1 = state[t]["araw0"], state[t]["araw1"]
            nkc = (t + 1) * 4
            h0, h1 = hp, hp + 7
            c0, r0 = h0 // 2, (h0 % 2) * 64
            c1, r1 = h1 // 2, (h1 % 2) * 64
            kt_h0 = kt if r0 == 0 else kt2
            kt_h1 = kt if r1 == 64 else kt2
            av0 = psav.tile([65, QT], F32, tag="av0", name=f"av0_{t}_{hp}")
            av1 = psav.tile([65, QT], F32, tag="av1", name=f"av1_{t}_{hp}")
            probs_q = []

            def emit_av(kc, probs):
                jsl = slice((kc - 4 * t) * P, QT) if kc >= 4 * t \
                    else slice(0, QT)
                nc.tensor.matmul(av0[:, jsl], vp[:, kc, 0:65],
                                 probs[:, 0, jsl],
                                 start=(kc == 0), stop=(kc == nkc - 1))
                nc.tensor.matmul(av1[:, jsl], vp[:, kc, 65:130],
                                 probs[:, 1, jsl],
                                 start=(kc == 0), stop=(kc == nkc - 1))

            for kc in range(nkc):
                ksl = slice(kc * P, (kc + 1) * P)
                diag = kc >= 4 * t
                j0 = kc - 4 * t if diag else 0
                jsl = slice(j0 * P, QT)      # causally reachable q columns
                st = pssc.tile([P, 2, QT], F32, tag="st",
                               name=f"st{t}_{hp}_{kc}")
                nc.tensor.matmul(st[:, 0, jsl], kt_h0[r0:r0 + 64, ksl],
                                 qs[r0:r0 + 64, c0, jsl],
                                 start=True, stop=True)
                if r0 == 0:
                    nc.tensor.matmul(st[:, 1, jsl], kt_h1[64:128, ksl],
                                     qs[64:128, c1, jsl],
                                     start=True, stop=True,
                                     tile_position=(64, 0))
                else:
                    nc.tensor.matmul(st[:, 1, jsl], kt_h1[0:64, ksl],
                                     qs[0:64, c1, jsl],
                                     start=True, stop=True)
                probs = ppool.tile([P, 2, QT], BF16, tag="probs",
                                   name=f"pr{t}_{hp}_{kc}")
                if j0 > 0:  # fully-masked q columns: zero instead of exp
                    nc.vector.memset(probs[:, :, 0:j0 * P], 0.0)
                nc.scalar.activation(probs[:, :, jsl], st[:, :, jsl],
                                     mybir.ActivationFunctionType.Exp,
                                     bias=bias_exp, scale=0.125)
                if diag:  # triangle block of the causal mask
                    tsl = slice(j0 * P, (j0 + 1) * P)
                    nc.gpsimd.affine_select(
                        out=probs[:, :, tsl], in_=probs[:, :, tsl],
                        pattern=[[0, 2], [1, P]],
                        compare_op=mybir.AluOpType.is_ge, fill=0.0,
                        base=0, channel_multiplier=-1)
                probs_q.append((kc, probs))
                if len(probs_q) > 1:
                    emit_av(*probs_q.pop(0))
                yield
            emit_av(*probs_q.pop(0))
            # evict raw accumulators (+denominator rows) as bf16
            nc.vector.tensor_copy(araw0[:, hp, :], av0)
            nc.vector.tensor_copy(araw1[:, hp, :], av1)

        # ---- software-pipelined emission ----
        fillers = []

        def drive(n):
            advanced = 0
            for _ in range(n):
                while fillers:
                    try:
                        next(fillers[0])
                        fillers.append(fillers.pop(0))
                        advanced += 1
                        break
                    except StopIteration:
                        fillers.pop(0)
            return advanced

        def drain(g):
            for _ in g:
                pass

        # dummy full-array matmul: keeps the HAM clock gate warm when the
        # filler pool runs dry (result unused)
        _dn = [0]

        def dummy():
            _dn[0] += 1
            dmy = psc.tile([P, QT], F32, tag="proj", name=f"dmy{_dn[0]}")
            nc.tensor.matmul(dmy, ones_sb[:, 0:P], ones_sb,
                             start=True, stop=True)

        # prologue: K of tile 0 (V/K2 deferred) and two Q chunks of tile 0
        g0 = gen_kv(0, preload=(xs0, cos0, sin0))
        while not state.get(0, {}).get("k_done"):
            next(g0)
        gq = gen_q(0)
        while state[0]["qn"] < REQ_CHUNKS[0]:
            next(gq)

        for t in range(NQT):
            state[t]["aslab"] = work.tile([P, HC, QT], BF16, tag="aslab",
                                          name=f"aslab{t}")
            state[t]["araw0"] = work.tile([65, HC, QT], BF16, tag="araw0",
                                          bufs=1, name=f"araw0_{t}")
            state[t]["araw1"] = work.tile([65, HC, QT], BF16, tag="araw1",
                                          bufs=1, name=f"araw1_{t}")
            if t == 0:
                fillers.append(g0)      # deferred V + K2 of tile 0
            fillers.append(gq)          # rest of this tile's Q projection
            if t >= 1:
                fillers.append(gen_oproj(t - 1))
            if t + 1 < NQT:
                fillers.append(gen_kv(t + 1))
            for hp in range(7):
                while state[t]["qn"] < REQ_CHUNKS[hp]:
                    next(gq)
                for _ in attention_pair(t, hp):
                    if drive(2 if t == 0 else 1) == 0:
                        dummy()
                fillers.append(norm_pair(t, hp))
                if t == NQT - 1 and hp == 3:
                    fillers.append(gen_oproj(t))
            drive(1000)
            if t + 1 < NQT:
                gq = gen_q(t + 1)
                while state[t + 1]["qn"] < REQ_CHUNKS[0]:
                    next(gq)
    nc.compile()
    return nc


def _host_prep(hidden_states, position_ids, Wq, bq, Wk, bk, Wv, bv, Wo):
    """Build per-core input maps (host-side layout work)."""
    import ml_dtypes
    f32 = np.float32
    bf16 = ml_dtypes.bfloat16
    HALF = 32

    def chunked(w, dt=f32):  # [H, N] -> [P, HC, N]
        return np.ascontiguousarray(
            w.reshape(HC, P, -1).transpose(1, 0, 2)).astype(dt)

    wq_h = chunked(Wq)
    wk_h = chunked(Wk)
    swap = np.concatenate([np.arange(64, 128), np.arange(0, 64)])
    wk2_h = chunked(Wk[:, swap])
    wv_h = chunked(Wv)
    # Wo rows permuted: chunk hp partition p -> head (hp | hp+7), dim p%64
    perm = np.empty(H, np.int64)
    for hp in range(7):
        for p in range(P):
            h = hp if p < 64 else hp + 7
            perm[hp * P + p] = h * 64 + (p % 64)
    wo_h = chunked(Wo[perm], bf16)
    bq_h = bq.reshape(1, H).astype(f32)
    bk_h = bk.reshape(1, P).astype(f32)
    bk2_h = bk[swap].reshape(1, P).astype(f32)
    bv_h = bv.reshape(1, P).astype(f32)
    pneg = np.zeros((P, P), f32)
    for i in range(P):
        pneg[i, i ^ 32] = -1.0
    ones_h = np.ones((P, QT), f32)
    ident_h = np.eye(P, dtype=f32)

    inv_freq = (1.0 / (THETA ** (np.arange(0, HALF, dtype=np.float64) / HALF)))
    pidx = np.arange(P)
    sign = np.where((pidx % 64) >= HALF, 1.0, -1.0)[:, None]

    in_maps = []
    for b in range(B):
        xt = np.ascontiguousarray(
            hidden_states[b].T.reshape(HC, P, S).transpose(1, 0, 2)).astype(f32)
        ang = position_ids[b].astype(np.float64)[None, :] * \
            inv_freq[pidx % HALF][:, None]          # [P, S]
        cos4 = np.cos(ang).astype(f32)
        sinm4 = (np.sin(ang) * sign).astype(f32)
        in_maps.append({
            "xt": xt, "wq": wq_h, "wk": wk_h, "wk2": wk2_h, "wv": wv_h,
            "wo": wo_h, "bq": bq_h, "bk": bk_h, "bk2": bk2_h, "bv": bv_h,
            "cos4": cos4, "sinm4": sinm4, "pneg": pneg, "onesr": ones_h,
            "ident": ident_h,
        })
    return in_maps


def kernel(**inputs):
    global LAST_RESULTS
    if "nc" not in _CACHE:
        _CACHE["nc"] = _build()
    nc = _CACHE["nc"]
    in_maps = _host_prep(**inputs)
    trace = bool(int(os.environ.get("KERNEL_TRACE", "0")))
    res = bass_utils.run_bass_kernel_spmd(
        nc, in_maps, core_ids=list(range(8)), trace=trace)
    LAST_RESULTS = res
    out = np.empty((B, S, H), np.float32)
    for b in range(B):
        o = res.results[b]["o"]              # [P, SC, H]
        out[b] = o.transpose(1, 0, 2).reshape(S, H)
    return out


# revision 23
# speedup vs baseline: 1.0598x; 1.0598x over previous
"""CosyVoice2 attention (B=8, S=2048, H=896, 14Q/2KV GQA, RoPE, causal) as a
Trainium2 Bass/Tile kernel, data-parallel over batch across 8 NeuronCores.

Per-core program (one batch element per core, no collectives):
  - host supplies X^T [896, 2048] (hidden on partitions, 7 chunks of 128),
    weights in matmul-ready layouts, and RoPE cos/sin tables with the
    rotate-half sign folded in.
  - QKV projections run as fp32r matmuls (full PE rate at N>=256) with
    biases added by K=1 ones-row matmuls into the same PSUM group.
  - RoPE in [d, s] layout:  out = x.*cos4 + Pneg @ (x.*sinm4), where
    Pneg[i, i^32] = -1.
  - scores S^T[k, q] per (head-pair, k-chunk): two K=64 matmuls row-tiled to
    opposite PE array halves run concurrently (heads h and h+7).
  - softmax: one ACT exp per k-chunk ([128, 2, 512] PSUM pair -> bf16
    probs), scale=1/8 and a constant -4 bias folded in.  Diagonal chunks
    restrict the exp/scores/attnV access patterns to causally-reachable q
    columns; fully-masked columns are zeroed by a DVE memset and the
    triangle block gets a gpsimd affine_select.
  - attnV in bf16 with the denominator riding as a 65th ones-row.  At pair
    end the raw [65, 512] accumulators are evicted to bf16 SBUF slabs;
    normalization (denom broadcast matmul -> fast reciprocal -> DVE mul
    into the bf16 A^T slab) and the bf16 o_proj run as filler in the NEXT
    q-tile's attention span, so the pair boundary frees PSUM in two DVE
    copies instead of a long normalize chain.
  - the chunk loop is software-pipelined (scores/exp of chunk k+1 emitted
    before attnV of chunk k) with projection / normalize / o_proj work
    interleaved as filler so the PE stays dense and the HAM clock gate
    stays un-throttled.
"""

import os
import sys

for _p in ("/opt/trn_rl_repo", "/root/.axon_site/_ro/trn_rl_repo"):
    if _p not in sys.path and os.path.isdir(_p):
        sys.path.append(_p)

import contextlib

import numpy as np

import concourse.bacc as bacc
import concourse.mybir as mybir
import concourse.tile as tile
from concourse import bass_utils

B = 8
S = 2048
H = 896
NQ = 14
NKV = 2
D = 64
THETA = 1000000.0
P = 128
HC = H // P          # 7 hidden chunks
QT = 512             # q-tile width
NQT = S // QT        # 4 q-tiles
SC = S // P          # 16 seq chunks of 128
F32 = mybir.dt.float32
F32R = mybir.dt.float32r
BF16 = mybir.dt.bfloat16

# order in which Q head-dim chunks are projected; pair hp needs chunks
# {hp//2, (hp+7)//2}, so this order lets pair p start after the first
# REQ_CHUNKS[p] entries have been emitted.
Q_ORDER = [0, 3, 4, 1, 5, 2, 6]
REQ_CHUNKS = [2, 3, 4, 5, 6, 7, 7]

_CACHE = {}
LAST_RESULTS = None


def _build():
    nc = bacc.Bacc("TRN2", target_bir_lowering=False, debug=False, num_devices=8)

    xt_d = nc.dram_tensor("xt", [P, HC, S], F32R, kind="ExternalInput").ap()
    wq_d = nc.dram_tensor("wq", [P, HC, H], F32R, kind="ExternalInput").ap()
    wk_d = nc.dram_tensor("wk", [P, HC, P], F32R, kind="ExternalInput").ap()
    wk2_d = nc.dram_tensor("wk2", [P, HC, P], F32R, kind="ExternalInput").ap()
    wv_d = nc.dram_tensor("wv", [P, HC, P], F32R, kind="ExternalInput").ap()
    wo_d = nc.dram_tensor("wo", [P, HC, H], BF16, kind="ExternalInput").ap()
    bq_d = nc.dram_tensor("bq", [1, H], F32R, kind="ExternalInput").ap()
    bk_d = nc.dram_tensor("bk", [1, P], F32R, kind="ExternalInput").ap()
    bk2_d = nc.dram_tensor("bk2", [1, P], F32R, kind="ExternalInput").ap()
    bv_d = nc.dram_tensor("bv", [1, P], F32R, kind="ExternalInput").ap()
    cos_d = nc.dram_tensor("cos4", [P, S], F32R, kind="ExternalInput").ap()
    sin_d = nc.dram_tensor("sinm4", [P, S], F32R, kind="ExternalInput").ap()
    pneg_d = nc.dram_tensor("pneg", [P, P], F32R, kind="ExternalInput").ap()
    ones_d = nc.dram_tensor("onesr", [P, QT], F32R, kind="ExternalInput").ap()
    ident_d = nc.dram_tensor("ident", [P, P], F32R, kind="ExternalInput").ap()
    o_d = nc.dram_tensor("o", [P, SC, H], F32, kind="ExternalOutput").ap()

    with tile.TileContext(nc) as tc, contextlib.ExitStack() as ctx:
        const = ctx.enter_context(tc.tile_pool(name="const", bufs=1))
        work = ctx.enter_context(tc.tile_pool(name="work", bufs=2))
        ppool = ctx.enter_context(tc.tile_pool(name="ppool", bufs=3))
        rpool = ctx.enter_context(tc.tile_pool(name="rpool", bufs=2))
        npool = ctx.enter_context(tc.tile_pool(name="npool", bufs=3))
        psc = ctx.enter_context(tc.tile_pool(name="psc", bufs=2, space="PSUM"))
        pssc = ctx.enter_context(tc.tile_pool(name="pssc", bufs=2, space="PSUM"))
        psav = ctx.enter_context(tc.tile_pool(name="psav", bufs=1, space="PSUM"))

        # ---- tile-0 activations first on the DMA queue, weights in order
        # of first use, wo (the biggest, needed only in span 1) last ----
        xs0 = work.tile([P, HC, QT], F32R, tag="xs", name="xs0")
        cos0 = work.tile([P, QT], F32R, tag="cos_t", name="cos0")
        sin0 = work.tile([P, QT], F32R, tag="sin_t", name="sin0")

        wk_sb = const.tile([P, HC, P], F32R)
        wk2_sb = const.tile([P, HC, P], F32R)
        wv_sb = const.tile([P, HC, P], F32R)
        wo_sb = const.tile([P, HC, H], BF16)
        bq_sb = const.tile([1, H], F32R)
        bk_sb = const.tile([1, P], F32R)
        bk2_sb = const.tile([1, P], F32R)
        bv_sb = const.tile([1, P], F32R)
        pneg_sb = const.tile([P, P], F32R)
        ident_sb = const.tile([P, P], F32R)
        ones_sb = const.tile([P, QT], F32R)
        ones16 = const.tile([P, 64], BF16)
        bias_exp = const.tile([P, 1], F32)
        # first K-proj matmul needs only xs chunks 0-1 + wk; stream those
        # first, the rest in order of first use
        nc.sync.dma_start(out=xs0[:, 0:2, :], in_=xt_d[:, 0:2, 0:QT])
        nc.sync.dma_start(out=wk_sb, in_=wk_d)
        nc.sync.dma_start(out=bk_sb, in_=bk_d)
        nc.sync.dma_start(out=cos0, in_=cos_d[:, 0:QT])
        nc.sync.dma_start(out=sin0, in_=sin_d[:, 0:QT])
        nc.sync.dma_start(out=xs0[:, 2:HC, :], in_=xt_d[:, 2:HC, 0:QT])
        for dst, src in ((pneg_sb, pneg_d), (ones_sb, ones_d),
                         (bq_sb, bq_d), (wv_sb, wv_d), (bv_sb, bv_d),
                         (ident_sb, ident_d),
                         (wk2_sb, wk2_d), (bk2_sb, bk2_d), (wo_sb, wo_d)):
            nc.sync.dma_start(out=dst, in_=src)
        nc.vector.memset(bias_exp, -4.0)
        nc.vector.tensor_copy(ones16, ones_sb[:, 0:64])

        # K^T resident (two partition layouts) and V' resident
        kt = const.tile([P, S], F32R)    # parts 0-63 = kv0, 64-127 = kv1
        kt2 = const.tile([P, S], F32R)   # parts 0-63 = kv1, 64-127 = kv0
        vp = const.tile([P, SC, 130], BF16)  # [Vkv0 | ones | Vkv1 | ones]
        nc.vector.memset(vp[:, :, 64:65], 1.0)
        nc.vector.memset(vp[:, :, 129:130], 1.0)

        # tiny PE touches that pre-absorb weight-DMA waits; emitted right
        # before each weight's first real matmul use (fresh ring tile per
        # call so the proj slot is released promptly)
        _tn = [0]

        def touch(*tiles):
            _tn[0] += 1
            tch = psc.tile([1, 2], F32, tag="proj", name=f"tch{_tn[0]}")
            for t in tiles:
                ap = (t[0:1, 0, 0:1] if len(t.shape) == 3 else t[0:1, 0:1])
                if t.dtype != BF16:
                    ap = ap.bitcast(F32)
                nc.tensor.matmul(tch[:, 0:1], ap, ap, start=True, stop=True)

        def rope_into(dst_ap, src_psum, cos_t, sin_t, nm):
            """dst = src*cos4 + Pneg @ (src*sinm4); evicts psum with 1 read"""
            qe = rpool.tile([P, QT], F32R, tag="qe", name=f"qe_{nm}")
            nc.vector.tensor_copy(qe, src_psum)
            t1 = rpool.tile([P, QT], F32R, tag="t1", name=f"t1_{nm}")
            nc.vector.tensor_mul(t1, qe, cos_t)
            nc.vector.tensor_mul(qe, qe, sin_t)
            rp = psc.tile([P, QT], F32, tag="proj", name=f"rp_{nm}")
            nc.tensor.matmul(rp, pneg_sb, qe, start=True, stop=True)
            nc.vector.tensor_add(dst_ap, t1.bitcast(F32), rp)

        state = {}

        def gen_kv(t, preload=None):
            """DMAs + K (then deferred V, K2) projections for q-tile t."""
            tslice = slice(t * QT, (t + 1) * QT)
            if preload is not None:
                xs, cos_t, sin_t = preload
            else:
                xs = work.tile([P, HC, QT], F32R, tag="xs", name=f"xs{t}")
                nc.sync.dma_start(out=xs, in_=xt_d[:, :, tslice])
                cos_t = work.tile([P, QT], F32R, tag="cos_t", name=f"cos{t}")
                sin_t = work.tile([P, QT], F32R, tag="sin_t", name=f"sin{t}")
                nc.sync.dma_start(out=cos_t, in_=cos_d[:, tslice])
                nc.sync.dma_start(out=sin_t, in_=sin_d[:, tslice])
            qs = work.tile([P, HC, QT], F32R, tag="qs", name=f"qs{t}")
            state[t] = {"qs": qs, "xs": xs, "cos": cos_t, "sin": sin_t,
                        "qn": 0, "nn": 0, "k_done": False, "v_done": False,
                        "k2_done": False}

            def kproj(kdst, w_sb, b_sb, nm):
                kp = psc.tile([P, QT], F32, tag="proj", name=f"kp_{nm}")
                for c in range(HC):
                    nc.tensor.matmul(kp, w_sb[:, c, :], xs[:, c, :],
                                     start=(c == 0), stop=False)
                    if c == 3:
                        yield
                nc.tensor.matmul(kp, b_sb, ones_sb[0:1, :], start=False,
                                 stop=True)
                rope_into(kdst[:, tslice], kp, cos_t, sin_t, nm)
                yield

            if t == 0:
                touch(wk_sb, bk_sb, pneg_sb, ones_sb)
            yield from kproj(kt, wk_sb, bk_sb, f"k{t}")
            state[t]["k_done"] = True
            # V projection: V^T then PE-transpose per 128-chunk
            if t == 0:
                touch(wv_sb, bv_sb, ident_sb)
            vtp = psc.tile([P, QT], F32, tag="proj", name=f"vtp{t}")
            for c in range(HC):
                nc.tensor.matmul(vtp, wv_sb[:, c, :], xs[:, c, :],
                                 start=(c == 0), stop=False)
                if c == 3:
                    yield
            nc.tensor.matmul(vtp, bv_sb, ones_sb[0:1, :], start=False,
                             stop=True)
            vt_sb = rpool.tile([P, QT], F32R, tag="vt_sb", name=f"vt{t}")
            nc.vector.tensor_copy(vt_sb, vtp)
            for j in range(4):
                sc_i = t * 4 + j
                vtr = psc.tile([P, P], F32R, tag="proj", name=f"vtr{sc_i}")
                nc.tensor.transpose(vtr, vt_sb[:, j * P:(j + 1) * P], ident_sb)
                nc.vector.tensor_copy(vp[:, sc_i, 0:64], vtr[:, 0:64])
                nc.vector.tensor_copy(vp[:, sc_i, 65:129], vtr[:, 64:128])
                if j % 2 == 1:
                    yield
            state[t]["v_done"] = True
            if t == 0:
                touch(wk2_sb, bk2_sb)
            yield from kproj(kt2, wk2_sb, bk2_sb, f"k2{t}")
            state[t]["k2_done"] = True

        def gen_q(t):
            """Q projection + rope for tile t, head-dim chunks in Q_ORDER."""
            st_ = state[t]
            xs, cos_t, sin_t, qs = st_["xs"], st_["cos"], st_["sin"], st_["qs"]
            if t == 0:
                touch(bq_sb)
            for c in Q_ORDER:
                wq_c = work.tile([P, HC, P], F32R, tag="wq_c", bufs=2,
                                 name=f"wq{t}_{c}")
                nc.sync.dma_start(out=wq_c, in_=wq_d[:, :, c * P:(c + 1) * P])
                qp = psc.tile([P, QT], F32, tag="proj", name=f"qp{t}_{c}")
                for hcc in range(HC):
                    nc.tensor.matmul(qp, wq_c[:, hcc, :], xs[:, hcc, :],
                                     start=(hcc == 0), stop=False)
                    if hcc == 3:
                        yield
                nc.tensor.matmul(qp, bq_sb[:, c * P:(c + 1) * P],
                                 ones_sb[0:1, :], start=False, stop=True)
                rope_into(qs[:, c, :], qp, cos_t, sin_t, f"q{t}_{c}")
                st_["qn"] += 1
                yield

        def norm_pair(t, hp):
            """Normalize pair hp of tile t from its raw bf16 slabs."""
            araw0, araw1 = state[t]["araw0"], state[t]["araw1"]
            aslab = state[t]["aslab"]
            for araw, rh in ((araw0, 0), (araw1, 64)):
                nm = f"n{t}_{hp}_{rh}"
                bc = psc.tile([64, QT], F32, tag="proj", name=f"bc{nm}")
                nc.tensor.matmul(bc, ones16[64:65, 0:64],
                                 araw[64:65, hp, :], start=True, stop=True)
                rc = npool.tile([64, QT], F32, tag="rc", bufs=2,
                                name=f"r{nm}")
                nc.vector.reciprocal_approx_fast(rc, bc)
                nc.vector.tensor_mul(aslab[rh:rh + 64, hp, :],
                                     araw[0:64, hp, :], rc)
                yield
            state[t]["nn"] += 1

        def gen_oproj(t):
            """bf16 o_proj of tile t (aslab must be normalized first)."""
            aslab = state[t]["aslab"]
            if t == 0:
                touch(wo_sb)
            for j in range(4):
                sc_i = t * 4 + j
                jsl = slice(j * P, (j + 1) * P)
                for n0, nw in ((0, 512), (512, 384)):
                    op = psc.tile([P, 512], F32, tag="proj",
                                  name=f"op{sc_i}_{n0}")
                    for c in range(HC):
                        # aslab chunk c must be normalized (emitted) first
                        while state[t]["nn"] <= c:
                            yield
                        nc.tensor.matmul(op[:, 0:nw], aslab[:, c, jsl],
                                         wo_sb[:, c, n0:n0 + nw],
                                         start=(c == 0), stop=(c == HC - 1))
                        if c == 3:
                            yield
                    osb = npool.tile([P, 512], F32, tag="osb", bufs=2,
                                     name=f"os{sc_i}_{n0}")
                    nc.vector.tensor_copy(osb[:, 0:nw], op[:, 0:nw])
                    nc.sync.dma_start(out=o_d[:, sc_i, n0:n0 + nw],
                                      in_=osb[:, 0:nw])
                    yield

        def attention_pair(t, hp):
            """Chunk-pipelined scores/softmax/attnV for head pair (hp, hp+7).

            Yields once per k-chunk; attnV matmuls for chunk kc are emitted
            after scores+exp of chunk kc+1 so the PE never waits on the ACT.
            Pair end evicts the raw accumulators (with denominator rows) to
            bf16 slabs; normalization happens next span.
            """
            qs = state[t]["qs"]
            araw0, araw1 = state[t]["araw0"], state[t]["araw1"]
            nkc = (t + 1) * 4
            h0, h1 = hp, hp + 7
            c0, r0 = h0 // 2, (h0 % 2) * 64
            c1, r1 = h1 // 2, (h1 % 2) * 64
            kt_h0 = kt if r0 == 0 else kt2
            kt_h1 = kt if r1 == 64 else kt2
            if kt_h0 is kt2 or kt_h1 is kt2:
                while not state[t]["k2_done"]:
                    yield       # let fillers finish the swapped-K projection
            av0 = psav.tile([65, QT], F32, tag="av0", name=f"av0_{t}_{hp}")
            av1 = psav.tile([65, QT], F32, tag="av1", name=f"av1_{t}_{hp}")
            probs_q = []

            def emit_av(kc, probs):
                jsl = slice((kc - 4 * t) * P, QT) if kc >= 4 * t \
                    else slice(0, QT)
                nc.tensor.matmul(av0[:, jsl], vp[:, kc, 0:65],
                                 probs[:, 0, jsl],
                                 start=(kc == 0), stop=(kc == nkc - 1))
                nc.tensor.matmul(av1[:, jsl], vp[:, kc, 65:130],
                                 probs[:, 1, jsl],
                                 start=(kc == 0), stop=(kc == nkc - 1))

            for kc in range(nkc):
                ksl = slice(kc * P, (kc + 1) * P)
                diag = kc >= 4 * t
                j0 = kc - 4 * t if diag else 0
                jsl = slice(j0 * P, QT)      # causally reachable q columns
                st = pssc.tile([P, 2, QT], F32, tag="st",
                               name=f"st{t}_{hp}_{kc}")
                nc.tensor.matmul(st[:, 0, jsl], kt_h0[r0:r0 + 64, ksl],
                                 qs[r0:r0 + 64, c0, jsl],
                                 start=True, stop=True)
                if r0 == 0:
                    nc.tensor.matmul(st[:, 1, jsl], kt_h1[64:128, ksl],
                                     qs[64:128, c1, jsl],
                                     start=True, stop=True,
                                     tile_position=(64, 0))
                else:
                    nc.tensor.matmul(st[:, 1, jsl], kt_h1[0:64, ksl],
                                     qs[0:64, c1, jsl],
                                     start=True, stop=True)
                probs = ppool.tile([P, 2, QT], BF16, tag="probs",
                                   name=f"pr{t}_{hp}_{kc}")
                if j0 > 0:  # fully-masked q columns: zero instead of exp
                    nc.vector.memset(probs[:, :, 0:j0 * P], 0.0)
                nc.scalar.activation(probs[:, :, jsl], st[:, :, jsl],
                                     mybir.ActivationFunctionType.Exp,
                                     bias=bias_exp, scale=0.125)
                if diag:  # triangle block of the causal mask
                    tsl = slice(j0 * P, (j0 + 1) * P)
                    nc.gpsimd.affine_select(
                        out=probs[:, :, tsl], in_=probs[:, :, tsl],
                        pattern=[[0, 2], [1, P]],
                        compare_op=mybir.AluOpType.is_ge, fill=0.0,
                        base=0, channel_multiplier=-1)
                probs_q.append((kc, probs))
                if len(probs_q) > 2:  # attnV lags scores/exp by 2 chunks
                    while not state[t]["v_done"]:
                        yield       # let fillers finish the V projection
                    emit_av(*probs_q.pop(0))
                yield
            while not state[t]["v_done"]:
                yield
            while probs_q:
                emit_av(*probs_q.pop(0))
            # evict raw accumulators (+denominator rows) as bf16
            nc.vector.tensor_copy(araw0[:, hp, :], av0)
            nc.vector.tensor_copy(araw1[:, hp, :], av1)

        # ---- software-pipelined emission ----
        fillers = []

        def drive(n):
            advanced = 0
            for _ in range(n):
                while fillers:
                    try:
                        next(fillers[0])
                        fillers.append(fillers.pop(0))
                        advanced += 1
                        break
                    except StopIteration:
                        fillers.pop(0)
            return advanced

        def drain(g):
            for _ in g:
                pass

        # dummy full-array matmul: keeps the HAM clock gate warm when the
        # filler pool runs dry (result unused)
        _dn = [0]

        def dummy():
            _dn[0] += 1
            dmy = psc.tile([P, QT], F32, tag="proj", name=f"dmy{_dn[0]}")
            nc.tensor.matmul(dmy, ones_sb[:, 0:P], ones_sb,
                             start=True, stop=True)

        # prologue: K of tile 0 (V/K2 deferred) and two Q chunks of tile 0
        g0 = gen_kv(0, preload=(xs0, cos0, sin0))
        while not state.get(0, {}).get("k_done"):
            next(g0)
        gq = gen_q(0)
        while state[0]["qn"] < REQ_CHUNKS[0]:
            next(gq)

        for t in range(NQT):
            state[t]["aslab"] = work.tile([P, HC, QT], BF16, tag="aslab",
                                          name=f"aslab{t}")
            state[t]["araw0"] = work.tile([65, HC, QT], BF16, tag="araw0",
                                          bufs=1, name=f"araw0_{t}")
            state[t]["araw1"] = work.tile([65, HC, QT], BF16, tag="araw1",
                                          bufs=1, name=f"araw1_{t}")
            if t == 0:
                fillers.append(g0)      # deferred V + K2 of tile 0
            fillers.append(gq)          # rest of this tile's Q projection
            if t >= 1:
                fillers.append(gen_oproj(t - 1))
            if t + 1 < NQT:
                fillers.append(gen_kv(t + 1))
            for hp in range(7):
                while state[t]["qn"] < REQ_CHUNKS[hp]:
                    next(gq)
                for _ in attention_pair(t, hp):
                    if drive(2 if t == 0 else 1) == 0:
                        dummy()
                fillers.append(norm_pair(t, hp))
                if t == NQT - 1 and hp == 3:
                    fillers.append(gen_oproj(t))
            drive(1000)
            if t + 1 < NQT:
                gq = gen_q(t + 1)
                while state[t + 1]["qn"] < REQ_CHUNKS[0]:
                    next(gq)
    nc.compile()
    return nc


def _host_prep(hidden_states, position_ids, Wq, bq, Wk, bk, Wv, bv, Wo):
    """Build per-core input maps (host-side layout work)."""
    import ml_dtypes
    f32 = np.float32
    bf16 = ml_dtypes.bfloat16
    HALF = 32

    def chunked(w, dt=f32):  # [H, N] -> [P, HC, N]
        return np.ascontiguousarray(
            w.reshape(HC, P, -1).transpose(1, 0, 2)).astype(dt)

    wq_h = chunked(Wq)
    wk_h = chunked(Wk)
    swap = np.concatenate([np.arange(64, 128), np.arange(0, 64)])
    wk2_h = chunked(Wk[:, swap])
    wv_h = chunked(Wv)
    # Wo rows permuted: chunk hp partition p -> head (hp | hp+7), dim p%64
    perm = np.empty(H, np.int64)
    for hp in range(7):
        for p in range(P):
            h = hp if p < 64 else hp + 7
            perm[hp * P + p] = h * 64 + (p % 64)
    wo_h = chunked(Wo[perm], bf16)
    bq_h = bq.reshape(1, H).astype(f32)
    bk_h = bk.reshape(1, P).astype(f32)
    bk2_h = bk[swap].reshape(1, P).astype(f32)
    bv_h = bv.reshape(1, P).astype(f32)
    pneg = np.zeros((P, P), f32)
    for i in range(P):
        pneg[i, i ^ 32] = -1.0
    ones_h = np.ones((P, QT), f32)
    ident_h = np.eye(P, dtype=f32)

    inv_freq = (1.0 / (THETA ** (np.arange(0, HALF, dtype=np.float64) / HALF)))
    pidx = np.arange(P)
    sign = np.where((pidx % 64) >= HALF, 1.0, -1.0)[:, None]

    in_maps = []
    for b in range(B):
        xt = np.ascontiguousarray(
            hidden_states[b].T.reshape(HC, P, S).transpose(1, 0, 2)).astype(f32)
        ang = position_ids[b].astype(np.float64)[None, :] * \
            inv_freq[pidx % HALF][:, None]          # [P, S]
        cos4 = np.cos(ang).astype(f32)
        sinm4 = (np.sin(ang) * sign).astype(f32)
        in_maps.append({
            "xt": xt, "wq": wq_h, "wk": wk_h, "wk2": wk2_h, "wv": wv_h,
            "wo": wo_h, "bq": bq_h, "bk": bk_h, "bk2": bk2_h, "bv": bv_h,
            "cos4": cos4, "sinm4": sinm4, "pneg": pneg, "onesr": ones_h,
            "ident": ident_h,
        })
    return in_maps


def kernel(**inputs):
    global LAST_RESULTS
    if "nc" not in _CACHE:
        _CACHE["nc"] = _build()
    nc = _CACHE["nc"]
    in_maps = _host_prep(**inputs)
    trace = bool(int(os.environ.get("KERNEL_TRACE", "0")))
    res = bass_utils.run_bass_kernel_spmd(
        nc, in_maps, core_ids=list(range(8)), trace=trace)
    LAST_RESULTS = res
    out = np.empty((B, S, H), np.float32)
    for b in range(B):
        o = res.results[b]["o"]              # [P, SC, H]
        out[b] = o.transpose(1, 0, 2).reshape(S, H)
    return out


# revision 25
# speedup vs baseline: 1.1302x; 1.0665x over previous
"""CosyVoice2 attention (B=8, S=2048, H=896, 14Q/2KV GQA, RoPE, causal) as a
Trainium2 Bass/Tile kernel, data-parallel over batch across 8 NeuronCores.

Per-core program (one batch element per core, no collectives):
  - host supplies X^T [896, 2048] (hidden on partitions, 7 chunks of 128),
    weights in matmul-ready layouts, and RoPE cos/sin tables with the
    rotate-half sign folded in.
  - QKV projections run as fp32r matmuls (full PE rate at N>=256) with
    biases added by K=1 ones-row matmuls into the same PSUM group.
  - RoPE in [d, s] layout:  out = x.*cos4 + Pneg @ (x.*sinm4), where
    Pneg[i, i^32] = -1.
  - scores S^T[k, q] per (head-pair, k-chunk): two K=64 matmuls row-tiled to
    opposite PE array halves run concurrently (heads h and h+7).
  - softmax: one ACT exp per k-chunk ([128, 2, 512] PSUM pair -> bf16
    probs), scale=1/8 and a constant -4 bias folded in.  Diagonal chunks
    restrict the exp/scores/attnV access patterns to causally-reachable q
    columns; fully-masked columns are zeroed by a DVE memset and the
    triangle block gets a gpsimd affine_select.
  - attnV in bf16 with the denominator riding as a 65th ones-row.  At pair
    end the raw [65, 512] accumulators are evicted to bf16 SBUF slabs;
    normalization (denom broadcast matmul -> fast reciprocal -> DVE mul
    into the bf16 A^T slab) and the bf16 o_proj run as filler in the NEXT
    q-tile's attention span, so the pair boundary frees PSUM in two DVE
    copies instead of a long normalize chain.
  - the chunk loop is software-pipelined (scores/exp of chunk k+1 emitted
    before attnV of chunk k) with projection / normalize / o_proj work
    interleaved as filler so the PE stays dense and the HAM clock gate
    stays un-throttled.
"""

import os
import sys

for _p in ("/opt/trn_rl_repo", "/root/.axon_site/_ro/trn_rl_repo"):
    if _p not in sys.path and os.path.isdir(_p):
        sys.path.append(_p)

import contextlib

import numpy as np

import concourse.bacc as bacc
import concourse.mybir as mybir
import concourse.tile as tile
from concourse import bass_utils

B = 8
S = 2048
H = 896
NQ = 14
NKV = 2
D = 64
THETA = 1000000.0
P = 128
HC = H // P          # 7 hidden chunks
QT = 512             # q-tile width
NQT = S // QT        # 4 q-tiles
SC = S // P          # 16 seq chunks of 128
F32 = mybir.dt.float32
F32R = mybir.dt.float32r
BF16 = mybir.dt.bfloat16

# order in which Q head-dim chunks are projected; pair hp needs chunks
# {hp//2, (hp+7)//2}, so this order lets pair p start after the first
# REQ_CHUNKS[p] entries have been emitted.
Q_ORDER = [0, 3, 4, 1, 5, 2, 6]
REQ_CHUNKS = [2, 3, 4, 5, 6, 7, 7]

_CACHE = {}
LAST_RESULTS = None


def _build():
    nc = bacc.Bacc("TRN2", target_bir_lowering=False, debug=False, num_devices=8)

    xt_d = nc.dram_tensor("xt", [P, HC, S], F32R, kind="ExternalInput").ap()
    wq_d = nc.dram_tensor("wq", [P, HC, H], F32R, kind="ExternalInput").ap()
    wk_d = nc.dram_tensor("wk", [P, HC, P], F32R, kind="ExternalInput").ap()
    wk2_d = nc.dram_tensor("wk2", [P, HC, P], F32R, kind="ExternalInput").ap()
    wv_d = nc.dram_tensor("wv", [P, HC, P], F32R, kind="ExternalInput").ap()
    wo_d = nc.dram_tensor("wo", [P, HC, H], BF16, kind="ExternalInput").ap()
    bq_d = nc.dram_tensor("bq", [1, H], F32R, kind="ExternalInput").ap()
    bk_d = nc.dram_tensor("bk", [1, P], F32R, kind="ExternalInput").ap()
    bk2_d = nc.dram_tensor("bk2", [1, P], F32R, kind="ExternalInput").ap()
    bv_d = nc.dram_tensor("bv", [1, P], F32R, kind="ExternalInput").ap()
    cos_d = nc.dram_tensor("cos4", [P, S], F32R, kind="ExternalInput").ap()
    sin_d = nc.dram_tensor("sinm4", [P, S], F32R, kind="ExternalInput").ap()
    pneg_d = nc.dram_tensor("pneg", [P, P], F32R, kind="ExternalInput").ap()
    ones_d = nc.dram_tensor("onesr", [P, QT], F32R, kind="ExternalInput").ap()
    ident_d = nc.dram_tensor("ident", [P, P], F32R, kind="ExternalInput").ap()
    o_d = nc.dram_tensor("o", [P, SC, H], F32, kind="ExternalOutput").ap()

    with tile.TileContext(nc) as tc, contextlib.ExitStack() as ctx:
        const = ctx.enter_context(tc.tile_pool(name="const", bufs=1))
        work = ctx.enter_context(tc.tile_pool(name="work", bufs=2))
        ppool = ctx.enter_context(tc.tile_pool(name="ppool", bufs=3))
        rpool = ctx.enter_context(tc.tile_pool(name="rpool", bufs=2))
        npool = ctx.enter_context(tc.tile_pool(name="npool", bufs=3))
        psc = ctx.enter_context(tc.tile_pool(name="psc", bufs=2, space="PSUM"))
        pssc = ctx.enter_context(tc.tile_pool(name="pssc", bufs=2, space="PSUM"))
        psav = ctx.enter_context(tc.tile_pool(name="psav", bufs=1, space="PSUM"))

        # ---- tile-0 activations first on the DMA queue, weights in order
        # of first use, wo (the biggest, needed only in span 1) last ----
        xs0 = work.tile([P, HC, QT], F32R, tag="xs", name="xs0")
        cos0 = work.tile([P, QT], F32R, tag="cos_t", name="cos0")
        sin0 = work.tile([P, QT], F32R, tag="sin_t", name="sin0")

        wk_sb = const.tile([P, HC, P], F32R)
        wk2_sb = const.tile([P, HC, P], F32R)
        wv_sb = const.tile([P, HC, P], F32R)
        wo_sb = const.tile([P, HC, H], BF16)
        bq_sb = const.tile([1, H], F32R)
        bk_sb = const.tile([1, P], F32R)
        bk2_sb = const.tile([1, P], F32R)
        bv_sb = const.tile([1, P], F32R)
        pneg_sb = const.tile([P, P], F32R)
        ident_sb = const.tile([P, P], F32R)
        ones_sb = const.tile([P, QT], F32R)
        ones16 = const.tile([P, 64], BF16)
        bias_exp = const.tile([P, 1], F32)
        # first K-proj matmul needs only xs chunks 0-1 + wk; stream those
        # first, the rest in order of first use
        nc.sync.dma_start(out=xs0[:, 0:2, :], in_=xt_d[:, 0:2, 0:QT])
        nc.sync.dma_start(out=wk_sb, in_=wk_d)
        nc.sync.dma_start(out=bk_sb, in_=bk_d)
        nc.sync.dma_start(out=cos0, in_=cos_d[:, 0:QT])
        nc.sync.dma_start(out=sin0, in_=sin_d[:, 0:QT])
        nc.sync.dma_start(out=xs0[:, 2:HC, :], in_=xt_d[:, 2:HC, 0:QT])
        for dst, src in ((pneg_sb, pneg_d), (ones_sb, ones_d),
                         (bq_sb, bq_d), (wv_sb, wv_d), (bv_sb, bv_d),
                         (ident_sb, ident_d),
                         (wk2_sb, wk2_d), (bk2_sb, bk2_d), (wo_sb, wo_d)):
            nc.sync.dma_start(out=dst, in_=src)
        nc.vector.memset(bias_exp, -4.0)
        nc.vector.tensor_copy(ones16, ones_sb[:, 0:64])

        # K^T resident (two partition layouts) and V' resident
        kt = const.tile([P, S], F32R)    # parts 0-63 = kv0, 64-127 = kv1
        kt2 = const.tile([P, S], F32R)   # parts 0-63 = kv1, 64-127 = kv0
        vp = const.tile([P, SC, 130], BF16)  # [Vkv0 | ones | Vkv1 | ones]
        nc.vector.memset(vp[:, :, 64:65], 1.0)
        nc.vector.memset(vp[:, :, 129:130], 1.0)

        # tiny PE touches that pre-absorb weight-DMA waits; emitted right
        # before each weight's first real matmul use (fresh ring tile per
        # call so the proj slot is released promptly)
        _tn = [0]

        def touch(*tiles):
            _tn[0] += 1
            tch = psc.tile([1, 2], F32, tag="proj", name=f"tch{_tn[0]}")
            for t in tiles:
                ap = (t[0:1, 0, 0:1] if len(t.shape) == 3 else t[0:1, 0:1])
                if t.dtype != BF16:
                    ap = ap.bitcast(F32)
                nc.tensor.matmul(tch[:, 0:1], ap, ap, start=True, stop=True)

        def rope_into(dst_ap, src_psum, cos_t, sin_t, nm):
            """dst = src*cos4 + Pneg @ (src*sinm4); evicts psum with 1 read"""
            qe = rpool.tile([P, QT], F32R, tag="qe", name=f"qe_{nm}")
            nc.vector.tensor_copy(qe, src_psum)
            t1 = rpool.tile([P, QT], F32R, tag="t1", name=f"t1_{nm}")
            nc.vector.tensor_mul(t1, qe, cos_t)
            nc.vector.tensor_mul(qe, qe, sin_t)
            rp = psc.tile([P, QT], F32, tag="proj", name=f"rp_{nm}")
            nc.tensor.matmul(rp, pneg_sb, qe, start=True, stop=True)
            nc.vector.tensor_add(dst_ap, t1.bitcast(F32), rp)

        state = {}

        def gen_kv(t, preload=None):
            """DMAs + K (then deferred V, K2) projections for q-tile t."""
            tslice = slice(t * QT, (t + 1) * QT)
            if preload is not None:
                xs, cos_t, sin_t = preload
            else:
                xs = work.tile([P, HC, QT], F32R, tag="xs", name=f"xs{t}")
                nc.sync.dma_start(out=xs, in_=xt_d[:, :, tslice])
                cos_t = work.tile([P, QT], F32R, tag="cos_t", name=f"cos{t}")
                sin_t = work.tile([P, QT], F32R, tag="sin_t", name=f"sin{t}")
                nc.sync.dma_start(out=cos_t, in_=cos_d[:, tslice])
                nc.sync.dma_start(out=sin_t, in_=sin_d[:, tslice])
            qs = work.tile([P, HC, QT], F32R, tag="qs", name=f"qs{t}")
            state[t] = {"qs": qs, "xs": xs, "cos": cos_t, "sin": sin_t,
                        "qn": 0, "nn": 0, "k_done": False, "v_done": False,
                        "k2_done": False}

            def kproj(kdst, w_sb, b_sb, nm):
                kp = psc.tile([P, QT], F32, tag="proj", name=f"kp_{nm}")
                for c in range(HC):
                    nc.tensor.matmul(kp, w_sb[:, c, :], xs[:, c, :],
                                     start=(c == 0), stop=False)
                    if c == 3:
                        yield
                nc.tensor.matmul(kp, b_sb, ones_sb[0:1, :], start=False,
                                 stop=True)
                rope_into(kdst[:, tslice], kp, cos_t, sin_t, nm)
                yield

            if t == 0:
                touch(wk_sb, bk_sb, pneg_sb, ones_sb)
            yield from kproj(kt, wk_sb, bk_sb, f"k{t}")
            state[t]["k_done"] = True
            # V projection: V^T then PE-transpose per 128-chunk
            if t == 0:
                touch(wv_sb, bv_sb, ident_sb)
            vtp = psc.tile([P, QT], F32, tag="proj", name=f"vtp{t}")
            for c in range(HC):
                nc.tensor.matmul(vtp, wv_sb[:, c, :], xs[:, c, :],
                                 start=(c == 0), stop=False)
                if c == 3:
                    yield
            nc.tensor.matmul(vtp, bv_sb, ones_sb[0:1, :], start=False,
                             stop=True)
            vt_sb = rpool.tile([P, QT], F32R, tag="vt_sb", name=f"vt{t}")
            nc.vector.tensor_copy(vt_sb, vtp)
            for j in range(4):
                sc_i = t * 4 + j
                vtr = psc.tile([P, P], F32R, tag="proj", name=f"vtr{sc_i}")
                nc.tensor.transpose(vtr, vt_sb[:, j * P:(j + 1) * P], ident_sb)
                nc.vector.tensor_copy(vp[:, sc_i, 0:64], vtr[:, 0:64])
                nc.vector.tensor_copy(vp[:, sc_i, 65:129], vtr[:, 64:128])
                if j % 2 == 1:
                    yield
            state[t]["v_done"] = True
            if t == 0:
                touch(wk2_sb, bk2_sb)
            yield from kproj(kt2, wk2_sb, bk2_sb, f"k2{t}")
            state[t]["k2_done"] = True

        def gen_q(t):
            """Q projection + rope for tile t, head-dim chunks in Q_ORDER."""
            st_ = state[t]
            xs, cos_t, sin_t, qs = st_["xs"], st_["cos"], st_["sin"], st_["qs"]
            if t == 0:
                touch(bq_sb)
            for c in Q_ORDER:
                wq_c = work.tile([P, HC, P], F32R, tag="wq_c", bufs=2,
                                 name=f"wq{t}_{c}")
                nc.sync.dma_start(out=wq_c, in_=wq_d[:, :, c * P:(c + 1) * P])
                qp = psc.tile([P, QT], F32, tag="proj", name=f"qp{t}_{c}")
                for hcc in range(HC):
                    nc.tensor.matmul(qp, wq_c[:, hcc, :], xs[:, hcc, :],
                                     start=(hcc == 0), stop=False)
                    if hcc == 3:
                        yield
                nc.tensor.matmul(qp, bq_sb[:, c * P:(c + 1) * P],
                                 ones_sb[0:1, :], start=False, stop=True)
                rope_into(qs[:, c, :], qp, cos_t, sin_t, f"q{t}_{c}")
                st_["qn"] += 1
                yield

        def norm_pair(t, hp):
            """Normalize pair hp of tile t from its raw bf16 slabs."""
            araw0, araw1 = state[t]["araw0"], state[t]["araw1"]
            aslab = state[t]["aslab"]
            for araw, rh in ((araw0, 0), (araw1, 64)):
                nm = f"n{t}_{hp}_{rh}"
                bc = psc.tile([64, QT], F32, tag="proj", name=f"bc{nm}")
                nc.tensor.matmul(bc, ones16[64:65, 0:64],
                                 araw[64:65, hp, :], start=True, stop=True)
                rc = npool.tile([64, QT], F32, tag="rc", bufs=2,
                                name=f"r{nm}")
                nc.vector.reciprocal_approx_fast(rc, bc)
                nc.vector.tensor_mul(aslab[rh:rh + 64, hp, :],
                                     araw[0:64, hp, :], rc)
                yield
            state[t]["nn"] += 1

        def gen_oproj(t):
            """bf16 o_proj of tile t (aslab must be normalized first)."""
            aslab = state[t]["aslab"]
            if t == 0:
                touch(wo_sb)
            for j in range(4):
                sc_i = t * 4 + j
                jsl = slice(j * P, (j + 1) * P)
                for n0, nw in ((0, 512), (512, 384)):
                    op = psc.tile([P, 512], F32, tag="proj",
                                  name=f"op{sc_i}_{n0}")
                    for c in range(HC):
                        # aslab chunk c must be normalized (emitted) first
                        while state[t]["nn"] <= c:
                            yield
                        nc.tensor.matmul(op[:, 0:nw], aslab[:, c, jsl],
                                         wo_sb[:, c, n0:n0 + nw],
                                         start=(c == 0), stop=(c == HC - 1))
                        if c == 3:
                            yield
                    osb = npool.tile([P, 512], F32, tag="osb", bufs=2,
                                     name=f"os{sc_i}_{n0}")
                    nc.vector.tensor_copy(osb[:, 0:nw], op[:, 0:nw])
                    nc.sync.dma_start(out=o_d[:, sc_i, n0:n0 + nw],
                                      in_=osb[:, 0:nw])
                    yield

        def attention_tile(t):
            """Chunk-pipelined scores/softmax/attnV for all 7 head pairs of
            q-tile t.  The attnV lag-2 queue crosses pair boundaries, so
            scores/exp of the next pair flow while the previous pair's tail
            attnV matmuls and bf16 evictions drain.  All input gates are
            cooperative yields (fillers advance while we wait)."""
            qs = state[t]["qs"]
            araw0, araw1 = state[t]["araw0"], state[t]["araw1"]
            nkc = (t + 1) * 4
            q = []   # (av0, av1, hp, kc, probs)

            def pop_emit():
                av0, av1, hp, kc, probs = q.pop(0)
                jsl = slice((kc - 4 * t) * P, QT) if kc >= 4 * t \
                    else slice(0, QT)
                nc.tensor.matmul(av0[:, jsl], vp[:, kc, 0:65],
                                 probs[:, 0, jsl],
                                 start=(kc == 0), stop=(kc == nkc - 1))
                nc.tensor.matmul(av1[:, jsl], vp[:, kc, 65:130],
                                 probs[:, 1, jsl],
                                 start=(kc == 0), stop=(kc == nkc - 1))
                if kc == nkc - 1:
                    # evict raw accumulators (+denominator rows) as bf16
                    nc.vector.tensor_copy(araw0[:, hp, :], av0)
                    nc.vector.tensor_copy(araw1[:, hp, :], av1)
                    fillers.append(norm_pair(t, hp))

            while not state[t]["k_done"]:
                yield
            for hp in range(7):
                while state[t]["qn"] < REQ_CHUNKS[hp]:
                    yield
                if hp == 1:
                    while not state[t]["k2_done"]:
                        yield
                h0, h1 = hp, hp + 7
                c0, r0 = h0 // 2, (h0 % 2) * 64
                c1, r1 = h1 // 2, (h1 % 2) * 64
                kt_h0 = kt if r0 == 0 else kt2
                kt_h1 = kt if r1 == 64 else kt2
                av0 = psav.tile([65, QT], F32, tag="av0", name=f"av0_{t}_{hp}")
                av1 = psav.tile([65, QT], F32, tag="av1", name=f"av1_{t}_{hp}")
                for kc in range(nkc):
                    ksl = slice(kc * P, (kc + 1) * P)
                    diag = kc >= 4 * t
                    j0 = kc - 4 * t if diag else 0
                    jsl = slice(j0 * P, QT)  # causally reachable q columns
                    st = pssc.tile([P, 2, QT], F32, tag="st",
                                   name=f"st{t}_{hp}_{kc}")
                    nc.tensor.matmul(st[:, 0, jsl], kt_h0[r0:r0 + 64, ksl],
                                     qs[r0:r0 + 64, c0, jsl],
                                     start=True, stop=True)
                    if r0 == 0:
                        nc.tensor.matmul(st[:, 1, jsl], kt_h1[64:128, ksl],
                                         qs[64:128, c1, jsl],
                                         start=True, stop=True,
                                         tile_position=(64, 0))
                    else:
                        nc.tensor.matmul(st[:, 1, jsl], kt_h1[0:64, ksl],
                                         qs[0:64, c1, jsl],
                                         start=True, stop=True)
                    probs = ppool.tile([P, 2, QT], BF16, tag="probs",
                                       name=f"pr{t}_{hp}_{kc}")
                    if j0 > 0:  # fully-masked q columns: zero instead of exp
                        nc.vector.memset(probs[:, :, 0:j0 * P], 0.0)
                    nc.scalar.activation(probs[:, :, jsl], st[:, :, jsl],
                                         mybir.ActivationFunctionType.Exp,
                                         bias=bias_exp, scale=0.125)
                    if diag:  # triangle block of the causal mask
                        tsl = slice(j0 * P, (j0 + 1) * P)
                        nc.gpsimd.affine_select(
                            out=probs[:, :, tsl], in_=probs[:, :, tsl],
                            pattern=[[0, 2], [1, P]],
                            compare_op=mybir.AluOpType.is_ge, fill=0.0,
                            base=0, channel_multiplier=-1)
                    q.append((av0, av1, hp, kc, probs))
                    if len(q) > 2:   # attnV lags scores/exp by 2 chunks
                        while not state[t]["v_done"]:
                            yield
                        pop_emit()
                    yield
                if t == NQT - 1 and hp == 3:
                    fillers.append(gen_oproj(t))
            while not state[t]["v_done"]:
                yield
            while q:
                pop_emit()

        # ---- software-pipelined emission ----
        fillers = []

        def drive(n):
            advanced = 0
            for _ in range(n):
                while fillers:
                    try:
                        next(fillers[0])
                        fillers.append(fillers.pop(0))
                        advanced += 1
                        break
                    except StopIteration:
                        fillers.pop(0)
            return advanced

        def drain(g):
            for _ in g:
                pass

        # dummy full-array matmul: keeps the HAM clock gate warm when the
        # filler pool runs dry (result unused)
        _dn = [0]

        def dummy():
            _dn[0] += 1
            dmy = psc.tile([P, QT], F32, tag="proj", name=f"dmy{_dn[0]}")
            nc.tensor.matmul(dmy, ones_sb[:, 0:P], ones_sb,
                             start=True, stop=True)

        # prologue: K of tile 0 (V/K2 deferred) and two Q chunks of tile 0
        g0 = gen_kv(0, preload=(xs0, cos0, sin0))
        while not state.get(0, {}).get("k_done"):
            next(g0)
        gq = gen_q(0)
        while state[0]["qn"] < REQ_CHUNKS[0]:
            next(gq)

        for t in range(NQT):
            state[t]["aslab"] = work.tile([P, HC, QT], BF16, tag="aslab",
                                          name=f"aslab{t}")
            state[t]["araw0"] = work.tile([65, HC, QT], BF16, tag="araw0",
                                          bufs=1, name=f"araw0_{t}")
            state[t]["araw1"] = work.tile([65, HC, QT], BF16, tag="araw1",
                                          bufs=1, name=f"araw1_{t}")
            if t == 0:
                fillers.append(g0)      # deferred V + K2 of tile 0
            if gq not in fillers:
                fillers.append(gq)      # rest of this tile's Q projection
            if t >= 1:
                fillers.append(gen_oproj(t - 1))
            if t + 1 < NQT:
                fillers.append(gen_kv(t + 1))
                gq_next = gen_q(t + 1)
                fillers.append(gq_next)
            for _ in attention_tile(t):
                if drive(2 if t == 0 else 1) == 0:
                    dummy()
            if t + 1 < NQT:
                gq = gq_next
        drive(5000)
    nc.compile()
    return nc


def _host_prep(hidden_states, position_ids, Wq, bq, Wk, bk, Wv, bv, Wo):
    """Build per-core input maps (host-side layout work)."""
    import ml_dtypes
    f32 = np.float32
    bf16 = ml_dtypes.bfloat16
    HALF = 32

    def chunked(w, dt=f32):  # [H, N] -> [P, HC, N]
        return np.ascontiguousarray(
            w.reshape(HC, P, -1).transpose(1, 0, 2)).astype(dt)

    wq_h = chunked(Wq)
    wk_h = chunked(Wk)
    swap = np.concatenate([np.arange(64, 128), np.arange(0, 64)])
    wk2_h = chunked(Wk[:, swap])
    wv_h = chunked(Wv)
    # Wo rows permuted: chunk hp partition p -> head (hp | hp+7), dim p%64
    perm = np.empty(H, np.int64)
    for hp in range(7):
        for p in range(P):
            h = hp if p < 64 else hp + 7
            perm[hp * P + p] = h * 64 + (p % 64)
    wo_h = chunked(Wo[perm], bf16)
    bq_h = bq.reshape(1, H).astype(f32)
    bk_h = bk.reshape(1, P).astype(f32)
    bk2_h = bk[swap].reshape(1, P).astype(f32)
    bv_h = bv.reshape(1, P).astype(f32)
    pneg = np.zeros((P, P), f32)
    for i in range(P):
        pneg[i, i ^ 32] = -1.0
    ones_h = np.ones((P, QT), f32)
    ident_h = np.eye(P, dtype=f32)

    inv_freq = (1.0 / (THETA ** (np.arange(0, HALF, dtype=np.float64) / HALF)))
    pidx = np.arange(P)
    sign = np.where((pidx % 64) >= HALF, 1.0, -1.0)[:, None]

    in_maps = []
    for b in range(B):
        xt = np.ascontiguousarray(
            hidden_states[b].T.reshape(HC, P, S).transpose(1, 0, 2)).astype(f32)
        ang = position_ids[b].astype(np.float64)[None, :] * \
            inv_freq[pidx % HALF][:, None]          # [P, S]
        cos4 = np.cos(ang).astype(f32)
        sinm4 = (np.sin(ang) * sign).astype(f32)
        in_maps.append({
            "xt": xt, "wq": wq_h, "wk": wk_h, "wk2": wk2_h, "wv": wv_h,
            "wo": wo_h, "bq": bq_h, "bk": bk_h, "bk2": bk2_h, "bv": bv_h,
            "cos4": cos4, "sinm4": sinm4, "pneg": pneg, "onesr": ones_h,
            "ident": ident_h,
        })
    return in_maps


def kernel(**inputs):
    global LAST_RESULTS
    if "nc" not in _CACHE:
        _CACHE["nc"] = _build()
    nc = _CACHE["nc"]
    in_maps = _host_prep(**inputs)
    trace = bool(int(os.environ.get("KERNEL_TRACE", "0")))
    res = bass_utils.run_bass_kernel_spmd(
        nc, in_maps, core_ids=list(range(8)), trace=trace)
    LAST_RESULTS = res
    out = np.empty((B, S, H), np.float32)
    for b in range(B):
        o = res.results[b]["o"]              # [P, SC, H]
        out[b] = o.transpose(1, 0, 2).reshape(S, H)
    return out
